# revision 1
# baseline (speedup 1.0000x reference)
"""Trainium2 Bass kernel for nn_LongAttention (gated linear attention).

Full inputs in, full outputs out. Internally: 8 NeuronCores, sequence-parallel
sharding (2 batches x 4 chunks of 1024 tokens). Per core, everything runs in a
[channel -> partition, time -> free] layout:

  - causal depthwise conv (4 taps) + SiLU on VectorE/ScalarE
  - q/k/v/i/o projections as bf16 matmuls (weights stationary, x^T streamed)
  - per-head L2/LN normalisation via ones-column matmul stats + gpsimd
    partition_broadcast
  - the gated recurrence mem_t = gamma_t * mem_{t-1} + kv_t runs on VectorE
    via tensor_tensor_scan (fp32 state) per head
  - cross-chunk scan carries are combined with one small AllGather (16 KB)
    and a prefix (Horner) combine on-chip; per-core one-hot `sel` picks the
    right prefix
  - epilogue: LN(mem), *q, GroupNorm, *sigmoid out-gate, then the Wo matmul;
    output returned transposed and reassembled on host.
"""

import numpy as np

B, T, C, H, K = 2, 4096, 2048, 16, 4
D = C // H
EPS = 1e-5
N_CORES = 8
TC = (B * T) // N_CORES          # tokens per core (1024)
NCT = C // 128                   # channel tiles (16)
HALO = K - 1                     # conv halo (3)
F32 = None                       # set after imports in _build
BF16 = None

_cache = {}


def _build_program():
    import concourse.bacc as bacc
    import concourse.mybir as mybir
    import concourse.tile as tile

    dt = mybir.dt
    AF = mybir.ActivationFunctionType
    OP = mybir.AluOpType

    nc = bacc.Bacc("TRN2", target_bir_lowering=False, debug=False,
                   num_devices=N_CORES)

    # ---- external inputs (per core) ----
    xT = nc.dram_tensor("xT", [C, TC + HALO], dt.bfloat16, kind="ExternalInput")
    wq = nc.dram_tensor("wq", [NCT, 128, NCT, 128], dt.bfloat16, kind="ExternalInput")
    wk = nc.dram_tensor("wk", [NCT, 128, NCT, 128], dt.bfloat16, kind="ExternalInput")
    wv = nc.dram_tensor("wv", [NCT, 128, NCT, 128], dt.bfloat16, kind="ExternalInput")
    wig = nc.dram_tensor("wig", [NCT, 128, NCT, 128], dt.bfloat16, kind="ExternalInput")
    wog = nc.dram_tensor("wog", [NCT, 128, NCT, 128], dt.bfloat16, kind="ExternalInput")
    wo = nc.dram_tensor("wo", [NCT, 128, NCT, 128], dt.bfloat16, kind="ExternalInput")
    wgam = nc.dram_tensor("wgam", [NCT, 128, H], dt.bfloat16, kind="ExternalInput")
    convw = nc.dram_tensor("convw", [NCT, 128, K], dt.float32, kind="ExternalInput")
    convb = nc.dram_tensor("convb", [128, NCT], dt.float32, kind="ExternalInput")
    bigc = nc.dram_tensor("bigc", [128, NCT], dt.float32, kind="ExternalInput")
    bogc = nc.dram_tensor("bogc", [128, NCT], dt.float32, kind="ExternalInput")
    gngc = nc.dram_tensor("gngc", [128, NCT], dt.float32, kind="ExternalInput")
    gnbc = nc.dram_tensor("gnbc", [128, NCT], dt.float32, kind="ExternalInput")
    vng = nc.dram_tensor("vng", [128, 1], dt.float32, kind="ExternalInput")
    vnb = nc.dram_tensor("vnb", [128, 1], dt.float32, kind="ExternalInput")
    mng = nc.dram_tensor("mng", [128, 1], dt.float32, kind="ExternalInput")
    mnb = nc.dram_tensor("mnb", [128, 1], dt.float32, kind="ExternalInput")
    bgam = nc.dram_tensor("bgam", [H, 1], dt.float32, kind="ExternalInput")
    sel = nc.dram_tensor("sel", [H, N_CORES], dt.float32, kind="ExternalInput")
    yT = nc.dram_tensor("yT", [C, TC], dt.float32, kind="ExternalOutput")

    SLAB = min(512, TC)
    NH = TC // SLAB  # matmul slabs per row

    with tile.TileContext(nc) as tc:
        with tc.tile_pool(name="big", bufs=1) as big, \
             tc.tile_pool(name="wp", bufs=3) as wp, \
             tc.tile_pool(name="rawp", bufs=9) as rawp, \
             tc.tile_pool(name="f32p", bufs=4) as f32p, \
             tc.tile_pool(name="bcp", bufs=4) as bcp, \
             tc.tile_pool(name="rowp", bufs=3) as rowp, \
             tc.tile_pool(name="ccp", bufs=5) as ccp, \
             tc.tile_pool(name="pp", bufs=2, space="PSUM") as pp, \
             tc.tile_pool(name="sp", bufs=2, space="PSUM") as sp, \
             tc.tile_pool(name="dram", bufs=1, space="DRAM") as dram:

            # ---- persistent SBUF tiles ----
            xts = big.tile([128, NCT, TC + HALO], dt.bfloat16, tag="bigA")
            xc = big.tile([128, NCT, TC], dt.bfloat16)
            cwsb = big.tile([128, NCT, K], dt.float32)
            convb_sb = big.tile([128, NCT], dt.float32)
            bigc_sb = big.tile([128, NCT], dt.float32)
            bogc_sb = big.tile([128, NCT], dt.float32)
            gngc_sb = big.tile([128, NCT], dt.float32)
            gnbc_sb = big.tile([128, NCT], dt.float32)
            vng_sb = big.tile([128, 1], dt.float32)
            vnb_sb = big.tile([128, 1], dt.float32)
            mng_sb = big.tile([128, 1], dt.float32)
            mnb_sb = big.tile([128, 1], dt.float32)
            bgam_sb = big.tile([H, 1], dt.float32)
            sel_sb = big.tile([H, N_CORES], dt.float32)
            wgam_sb = big.tile([128, NCT, H], dt.bfloat16)
            ones_bf = big.tile([128, 1], dt.bfloat16)
            ones16 = big.tile([H, TC], dt.bfloat16)
            gamma_sb = big.tile([H, TC], dt.float32)
            G_sb = big.tile([H, TC], dt.float32)

            eps1 = big.tile([1, 1], dt.float32)
            nc.gpsimd.memset(eps1[:], EPS)
            nc.gpsimd.memset(ones_bf[:], 1.0)
            nc.gpsimd.memset(ones16[:], 1.0)

            # DRAM scratch (per-core local spills) + collective bounce buffers
            qn_dram = dram.tile([NCT, 128, TC], dt.bfloat16)
            mem_dram = dram.tile([NCT, 128, TC], dt.bfloat16)
            og_dram = dram.tile([NCT, 128, TC], dt.bfloat16)
            ig_dram = dram.tile([NCT, 128, TC], dt.bfloat16)
            cc_in = dram.tile([1, 2 * C], dt.float32)
            cc_out = dram.tile([N_CORES, 2 * C], dt.float32, addr_space="Shared")
            s_dram = dram.tile([H, 128], dt.float32)

            # ---- load constants + x^T ----
            for i in range(NCT):
                nc.sync.dma_start(xts[:, i, :], xT[i * 128:(i + 1) * 128, :])
                nc.sync.dma_start(cwsb[:, i, :], convw[i])
                nc.sync.dma_start(wgam_sb[:, i, :], wgam[i])
            nc.sync.dma_start(convb_sb[:], convb[:])
            nc.sync.dma_start(bigc_sb[:], bigc[:])
            nc.sync.dma_start(bogc_sb[:], bogc[:])
            nc.sync.dma_start(gngc_sb[:], gngc[:])
            nc.sync.dma_start(gnbc_sb[:], gnbc[:])
            nc.sync.dma_start(vng_sb[:], vng[:])
            nc.sync.dma_start(vnb_sb[:], vnb[:])
            nc.sync.dma_start(mng_sb[:], mng[:])
            nc.sync.dma_start(mnb_sb[:], mnb[:])
            nc.sync.dma_start(bgam_sb[:], bgam[:])
            nc.sync.dma_start(sel_sb[:], sel[:])

            # ---- phase 1: causal depthwise conv + SiLU -> xc ----
            for i in range(NCT):
                a0 = rawp.tile([128, TC], dt.float32, tag="cacc", bufs=2)
                a1 = rawp.tile([128, TC], dt.float32, tag="cacc", bufs=2)
                nc.vector.tensor_scalar_mul(a0[:], xts[:, i, 0:TC], cwsb[:, i, 0:1])
                nc.vector.scalar_tensor_tensor(a1[:], xts[:, i, 1:1 + TC],
                                               cwsb[:, i, 1:2], a0[:],
                                               OP.mult, OP.add)
                nc.vector.scalar_tensor_tensor(a0[:], xts[:, i, 2:2 + TC],
                                               cwsb[:, i, 2:3], a1[:],
                                               OP.mult, OP.add)
                nc.vector.scalar_tensor_tensor(a1[:], xts[:, i, 3:3 + TC],
                                               cwsb[:, i, 3:4], a0[:],
                                               OP.mult, OP.add)
                sg = rawp.tile([128, TC], dt.bfloat16, tag="raw")
                nc.scalar.activation(sg[:], a1[:], AF.Sigmoid,
                                     bias=convb_sb[:, i:i + 1], scale=1.0)
                zb = rawp.tile([128, TC], dt.bfloat16, tag="raw")
                nc.scalar.activation(zb[:], a1[:], AF.Identity,
                                     bias=convb_sb[:, i:i + 1], scale=1.0)
                nc.vector.tensor_mul(xc[:, i, :], zb[:], sg[:])

            # ---- gamma / cumprod G (fp32) ----
            glog = pp.tile([H, TC], dt.float32, tag="pp")
            for k in range(NCT):
                for s in range(NH):
                    nc.tensor.matmul(glog[:, s * SLAB:(s + 1) * SLAB],
                                     wgam_sb[:, k, :],
                                     xc[:, k, s * SLAB:(s + 1) * SLAB],
                                     start=(k == 0), stop=(k == NCT - 1))
            nc.scalar.activation(gamma_sb[:], glog[:], AF.Sigmoid,
                                 bias=bgam_sb[:], scale=1.0)
            nc.vector.tensor_tensor_scan(G_sb[:], gamma_sb[:], ones16[:], 1.0,
                                         OP.mult, OP.mult)

            def proj_psum(wtensor, h, rhs_view, tag="pp"):
                ps = pp.tile([128, TC], dt.float32, tag=tag)
                wt = wp.tile([128, NCT, 128], dt.bfloat16, tag="wt")
                nc.sync.dma_start(wt[:], wtensor[h])
                for k in range(NCT):
                    for s in range(NH):
                        nc.tensor.matmul(ps[:, s * SLAB:(s + 1) * SLAB],
                                         wt[:, k, :], rhs_view(k, s),
                                         start=(k == 0), stop=(k == NCT - 1))
                return ps

            def x_rhs(k, s):
                return xts[:, k, HALO + s * SLAB:HALO + (s + 1) * SLAB]

            def xc_rhs(k, s):
                return xc[:, k, s * SLAB:(s + 1) * SLAB]

            def colsum(src_bf16):
                """sum over partitions via ones-column matmul -> [1, TC] psum"""
                row = sp.tile([1, TC], dt.float32, tag="sp")
                for s in range(NH):
                    nc.tensor.matmul(row[:, s * SLAB:(s + 1) * SLAB], ones_bf[:],
                                     src_bf16[:, s * SLAB:(s + 1) * SLAB],
                                     start=True, stop=True)
                return row

            def bcast_row(row_bf16):
                """[1, TC] bf16 row -> [128, TC] bf16 via gpsimd"""
                out = bcp.tile([128, TC], dt.bfloat16, tag="bc")
                nc.gpsimd.partition_broadcast(out[:], row_bf16[:])
                return out

            # ---- phase 2a: sigmoid gates (one ACT table set, dense PE) ----
            for h in range(H):
                psI = proj_psum(wig, h, xc_rhs)
                igt = rawp.tile([128, TC], dt.bfloat16, tag="raw")
                nc.scalar.activation(igt[:], psI[:], AF.Sigmoid,
                                     bias=bigc_sb[:, h:h + 1], scale=1.0)
                nc.sync.dma_start(ig_dram[h], igt[:])
                psO = proj_psum(wog, h, xc_rhs)
                ogt = rawp.tile([128, TC], dt.bfloat16, tag="raw")
                nc.scalar.activation(ogt[:], psO[:], AF.Sigmoid,
                                     bias=bogc_sb[:, h:h + 1], scale=1.0)
                nc.sync.dma_start(og_dram[h], ogt[:])

            # ---- phase 2b: k/v/q projections + stats + kv + scan ----
            # software-pipelined emission: head h's matmuls land in the PE
            # stream before head h-1's stats so PE never idles on DVE chains.
            def proj_block(h):
                psK = proj_psum(wk, h, x_rhs)
                kraw = rawp.tile([128, TC], dt.bfloat16, tag="raw",
                                 name=f"kraw{h}")
                nc.vector.tensor_copy(kraw[:], psK[:])
                psV = proj_psum(wv, h, x_rhs)
                vraw = rawp.tile([128, TC], dt.bfloat16, tag="raw",
                                 name=f"vraw{h}")
                nc.vector.tensor_copy(vraw[:], psV[:])
                psQ = proj_psum(wq, h, x_rhs)
                qraw = rawp.tile([128, TC], dt.bfloat16, tag="raw",
                                 name=f"qraw{h}")
                nc.vector.tensor_copy(qraw[:], psQ[:])
                return kraw, vraw, qraw

            def stats_block(h, kraw, vraw, qraw):
                # k l2 norm
                ksq = rawp.tile([128, TC], dt.bfloat16, tag="raw")
                nc.vector.tensor_mul(ksq[:], kraw[:], kraw[:])
                kssq = colsum(ksq)
                lnk = rowp.tile([1, TC], dt.float32, tag="row")
                nc.scalar.activation(lnk[:], kssq[:], AF.Ln, bias=0.0, scale=1.0)
                rk = rowp.tile([1, TC], dt.bfloat16, tag="rowb")
                nc.scalar.activation(rk[:], lnk[:], AF.Exp, bias=0.0, scale=-0.5)
                rkb = bcast_row(rk)
                kn = rawp.tile([128, TC], dt.bfloat16, tag="raw")
                nc.vector.tensor_mul(kn[:], kraw[:], rkb[:])

                # v layernorm
                vsum = colsum(vraw)
                vsq = rawp.tile([128, TC], dt.bfloat16, tag="raw")
                nc.vector.tensor_mul(vsq[:], vraw[:], vraw[:])
                vssq = colsum(vsq)
                mean = rowp.tile([1, TC], dt.float32, tag="row")
                nc.vector.tensor_scalar_mul(mean[:], vsum[:], 1.0 / D)
                msq = rowp.tile([1, TC], dt.float32, tag="row")
                nc.vector.tensor_mul(msq[:], mean[:], mean[:])
                var = rowp.tile([1, TC], dt.float32, tag="row")
                nc.vector.scalar_tensor_tensor(var[:], vssq[:], 1.0 / D, msq[:],
                                               OP.mult, OP.subtract)
                lnv = rowp.tile([1, TC], dt.float32, tag="row")
                nc.scalar.activation(lnv[:], var[:], AF.Ln, bias=eps1[:], scale=1.0)
                rv = rowp.tile([1, TC], dt.float32, tag="row")
                nc.scalar.activation(rv[:], lnv[:], AF.Exp, bias=0.0, scale=-0.5)
                mrv = rowp.tile([1, TC], dt.bfloat16, tag="rowb")
                nc.vector.tensor_mul(mrv[:], mean[:], rv[:])
                rvbf = rowp.tile([1, TC], dt.bfloat16, tag="rowb")
                nc.vector.tensor_copy(rvbf[:], rv[:])
                rvb = bcast_row(rvbf)
                mrvb = bcast_row(mrv)
                v1 = rawp.tile([128, TC], dt.bfloat16, tag="raw")
                nc.vector.tensor_mul(v1[:], vraw[:], rvb[:])
                v2 = rawp.tile([128, TC], dt.bfloat16, tag="raw")
                nc.vector.tensor_sub(v2[:], v1[:], mrvb[:])
                vn = rawp.tile([128, TC], dt.bfloat16, tag="raw")
                nc.scalar.activation(vn[:], v2[:], AF.Identity,
                                     bias=vnb_sb[:], scale=vng_sb[:])

                # kv = i * k_n * v_n  (fp32 for the scan)
                igt = rawp.tile([128, TC], dt.bfloat16, tag="raw")
                nc.sync.dma_start(igt[:], ig_dram[h])
                kv1 = rawp.tile([128, TC], dt.bfloat16, tag="raw")
                nc.vector.tensor_mul(kv1[:], kn[:], vn[:])
                kvh = f32p.tile([128, TC], dt.float32, tag="f32t")
                nc.vector.tensor_mul(kvh[:], kv1[:], igt[:])

                # q l2 norm -> DRAM
                qsq = rawp.tile([128, TC], dt.bfloat16, tag="raw")
                nc.vector.tensor_mul(qsq[:], qraw[:], qraw[:])
                qssq = colsum(qsq)
                lnq = rowp.tile([1, TC], dt.float32, tag="row")
                nc.scalar.activation(lnq[:], qssq[:], AF.Ln, bias=0.0, scale=1.0)
                rq = rowp.tile([1, TC], dt.bfloat16, tag="rowb")
                nc.scalar.activation(rq[:], lnq[:], AF.Exp, bias=0.0, scale=-0.5)
                rqb = bcast_row(rq)
                qn = rawp.tile([128, TC], dt.bfloat16, tag="raw")
                nc.vector.tensor_mul(qn[:], qraw[:], rqb[:])
                nc.sync.dma_start(qn_dram[h], qn[:])

                # the scan: mem = gamma * mem + kv  (fp32 state)
                gam0 = rowp.tile([1, TC], dt.float32, tag="row")
                nc.sync.dma_start(gam0[:], gamma_sb[h:h + 1, :])
                gb = f32p.tile([128, TC], dt.float32, tag="f32t")
                nc.gpsimd.partition_broadcast(gb[:], gam0[:])
                smem = f32p.tile([128, TC], dt.float32, tag="f32t")
                nc.vector.tensor_tensor_scan(smem[:], gb[:], kvh[:], 0.0,
                                             OP.mult, OP.add)
                memb = rawp.tile([128, TC], dt.bfloat16, tag="raw")
                nc.vector.tensor_copy(memb[:], smem[:])
                nc.sync.dma_start(mem_dram[h], memb[:])

                # carry summary: A = cumprod(gamma) col, B = final state col
                a0 = rowp.tile([1, 1], dt.float32, tag="acol0", bufs=2)
                nc.sync.dma_start(a0[:], G_sb[h:h + 1, TC - 1:TC])
                acol = rowp.tile([128, 1], dt.float32, tag="acol", bufs=2)
                nc.gpsimd.partition_broadcast(acol[:], a0[:])
                nc.sync.dma_start(cc_in[0:1, h * 128:(h + 1) * 128], acol[:])
                nc.sync.dma_start(cc_in[0:1, C + h * 128:C + (h + 1) * 128],
                                  smem[:, TC - 1:TC])

            prev = proj_block(0)
            for h in range(1, H):
                cur = proj_block(h)
                stats_block(h - 1, *prev)
                prev = cur
            stats_block(H - 1, *prev)

            # ---- phase 4: all-gather carries + prefix combine ----
            nc.gpsimd.collective_compute(
                "AllGather", OP.bypass,
                replica_groups=[list(range(N_CORES))],
                ins=[cc_in[:]], outs=[cc_out[:]],
            )
            srun = ccp.tile([H, 128], dt.float32, tag="cc")
            ssel = ccp.tile([H, 128], dt.float32, tag="cc")
            nc.gpsimd.memset(srun[:], 0.0)
            nc.gpsimd.memset(ssel[:], 0.0)
            for j in range(N_CORES):
                ssel2 = ccp.tile([H, 128], dt.float32, tag="cc")
                nc.vector.scalar_tensor_tensor(ssel2[:], srun[:],
                                               sel_sb[:, j:j + 1], ssel[:],
                                               OP.mult, OP.add)
                ssel = ssel2
                if j == N_CORES - 1:
                    break
                if j == (N_CORES // 2) - 1:
                    srun = ccp.tile([H, 128], dt.float32, tag="cc")
                    nc.gpsimd.memset(srun[:], 0.0)
                else:
                    arow = ccp.tile([H, 128], dt.float32, tag="cc")
                    brow = ccp.tile([H, 128], dt.float32, tag="cc")
                    nc.sync.dma_start(arow[:], cc_out[j:j + 1, 0:C])
                    nc.sync.dma_start(brow[:], cc_out[j:j + 1, C:2 * C])
                    tmp = ccp.tile([H, 128], dt.float32, tag="cc")
                    nc.vector.tensor_mul(tmp[:], arow[:], srun[:])
                    nc.vector.tensor_add(tmp[:], tmp[:], brow[:])
                    srun = tmp
            nc.sync.dma_start(s_dram[:], ssel[:])

            scol = big.tile([128, H], dt.float32)
            for h in range(H):
                nc.sync.dma_start(scol[:, h:h + 1], s_dram[h:h + 1, :])

            # ---- phase 5: fixup + LN + *q + GroupNorm + *og -> outsb ----
            outsb = big.tile([128, NCT, TC], dt.bfloat16, tag="bigA")
            for h in range(H):
                memb = rawp.tile([128, TC], dt.bfloat16, tag="raw")
                nc.sync.dma_start(memb[:], mem_dram[h])
                g0 = ccp.tile([1, TC], dt.float32, tag="cc")
                nc.sync.dma_start(g0[:], G_sb[h:h + 1, :])
                gbG = f32p.tile([128, TC], dt.float32, tag="f32t")
                nc.gpsimd.partition_broadcast(gbG[:], g0[:])
                memf = rawp.tile([128, TC], dt.bfloat16, tag="raw")
                nc.vector.scalar_tensor_tensor(memf[:], gbG[:], scol[:, h:h + 1],
                                               memb[:], OP.mult, OP.add)

                def ln_rows(src_bf):
                    ssum = colsum(src_bf)
                    sq = rawp.tile([128, TC], dt.bfloat16, tag="raw")
                    nc.vector.tensor_mul(sq[:], src_bf[:], src_bf[:])
                    sssq = colsum(sq)
                    mean = ccp.tile([1, TC], dt.float32, tag="cc")
                    nc.vector.tensor_scalar_mul(mean[:], ssum[:], 1.0 / D)
                    msq = ccp.tile([1, TC], dt.float32, tag="cc")
                    nc.vector.tensor_mul(msq[:], mean[:], mean[:])
                    var = ccp.tile([1, TC], dt.float32, tag="cc")
                    nc.vector.scalar_tensor_tensor(var[:], sssq[:], 1.0 / D,
                                                   msq[:], OP.mult, OP.subtract)
                    lnr = ccp.tile([1, TC], dt.float32, tag="cc")
                    nc.scalar.activation(lnr[:], var[:], AF.Ln, bias=eps1[:], scale=1.0)
                    rstd = ccp.tile([1, TC], dt.float32, tag="cc")
                    nc.scalar.activation(rstd[:], lnr[:], AF.Exp, bias=0.0, scale=-0.5)
                    mr = rowp.tile([1, TC], dt.bfloat16, tag="rowb")
                    nc.vector.tensor_mul(mr[:], mean[:], rstd[:])
                    rstdb = rowp.tile([1, TC], dt.bfloat16, tag="rowb")
                    nc.vector.tensor_copy(rstdb[:], rstd[:])
                    return bcast_row(rstdb), bcast_row(mr)

                rstdb, mrb = ln_rows(memf)
                m1 = rawp.tile([128, TC], dt.bfloat16, tag="raw")
                nc.vector.tensor_mul(m1[:], memf[:], rstdb[:])
                m2 = rawp.tile([128, TC], dt.bfloat16, tag="raw")
                nc.vector.tensor_sub(m2[:], m1[:], mrb[:])
                memn = rawp.tile([128, TC], dt.bfloat16, tag="raw")
                nc.scalar.activation(memn[:], m2[:], AF.Identity,
                                     bias=mnb_sb[:], scale=mng_sb[:])

                qnh = rawp.tile([128, TC], dt.bfloat16, tag="raw")
                nc.sync.dma_start(qnh[:], qn_dram[h])
                p = rawp.tile([128, TC], dt.bfloat16, tag="raw")
                nc.vector.tensor_mul(p[:], memn[:], qnh[:])

                rstdb2, mrb2 = ln_rows(p)
                g1 = rawp.tile([128, TC], dt.bfloat16, tag="raw")
                nc.vector.tensor_mul(g1[:], p[:], rstdb2[:])
                g2 = rawp.tile([128, TC], dt.bfloat16, tag="raw")
                nc.vector.tensor_sub(g2[:], g1[:], mrb2[:])
                outn = rawp.tile([128, TC], dt.bfloat16, tag="raw")
                nc.scalar.activation(outn[:], g2[:], AF.Identity,
                                     bias=gnbc_sb[:, h:h + 1],
                                     scale=gngc_sb[:, h:h + 1])

                ogt = rawp.tile([128, TC], dt.bfloat16, tag="raw")
                nc.sync.dma_start(ogt[:], og_dram[h])
                nc.vector.tensor_mul(outsb[:, h, :], outn[:], ogt[:])

            # ---- phase 6: Wo matmul -> yT ----
            for oc in range(NCT):
                ps = pp.tile([128, TC], dt.float32, tag="pp")
                wt = wp.tile([128, NCT, 128], dt.bfloat16, tag="wt")
                nc.sync.dma_start(wt[:], wo[oc])
                for k in range(NCT):
                    for s in range(NH):
                        nc.tensor.matmul(ps[:, s * SLAB:(s + 1) * SLAB],
                                         wt[:, k, :],
                                         outsb[:, k, s * SLAB:(s + 1) * SLAB],
                                         start=(k == 0), stop=(k == NCT - 1))
                ysb = rawp.tile([128, TC], dt.float32, tag="ysb", bufs=1)
                nc.scalar.copy(ysb[:], ps[:])
                nc.sync.dma_start(yT[oc * 128:(oc + 1) * 128, :], ysb[:])

    nc.finalize()
    return nc


def _host_prep(inputs):
    import ml_dtypes
    bf16 = ml_dtypes.bfloat16

    def wtiles(w):
        # per-head stationary block [oc, p, k, c]: w_sb[p, k, c] = W.T[k*128+p, oc*128+c]
        wT = np.ascontiguousarray(w.T.astype(bf16))          # [C_in, C_out]
        return np.ascontiguousarray(
            wT.reshape(NCT, 128, NCT, 128).transpose(2, 1, 0, 3))

    def colmat(v):
        return np.ascontiguousarray(np.asarray(v, np.float32).reshape(NCT, 128).T)

    x = np.asarray(inputs["x"])
    common = dict(
        wq=wtiles(np.asarray(inputs["Wq"])),
        wk=wtiles(np.asarray(inputs["Wk"])),
        wv=wtiles(np.asarray(inputs["Wv"])),
        wig=wtiles(np.asarray(inputs["Wig"])),
        wog=wtiles(np.asarray(inputs["Wog"])),
        wo=wtiles(np.asarray(inputs["Wo"])),
        wgam=np.ascontiguousarray(
            np.asarray(inputs["Wgam"]).T.astype(bf16).reshape(NCT, 128, H)),
        convw=np.ascontiguousarray(
            np.asarray(inputs["conv_w"])[:, 0, :].astype(np.float32).reshape(NCT, 128, K)),
        convb=colmat(inputs["conv_b"]),
        bigc=colmat(inputs["big"]),
        bogc=colmat(inputs["bog"]),
        gngc=colmat(inputs["gn_g"]),
        gnbc=colmat(inputs["gn_b"]),
        vng=np.asarray(inputs["vn_g"], np.float32).reshape(128, 1),
        vnb=np.asarray(inputs["vn_b"], np.float32).reshape(128, 1),
        mng=np.asarray(inputs["mn_g"], np.float32).reshape(128, 1),
        mnb=np.asarray(inputs["mn_b"], np.float32).reshape(128, 1),
        bgam=np.asarray(inputs["bgam"], np.float32).reshape(H, 1),
    )

    xbf = x.astype(bf16)
    in_maps = []
    for core in range(N_CORES):
        b, j = divmod(core, N_CORES // B)
        lo = j * TC
        chunk = np.zeros((TC + HALO, C), bf16)
        src_lo = max(0, lo - HALO)
        chunk[HALO - (lo - src_lo):] = xbf[b, src_lo:lo + TC]
        selv = np.zeros((H, N_CORES), np.float32)
        selv[:, core] = 1.0
        m = dict(common)
        m["xT"] = np.ascontiguousarray(chunk.T)
        m["sel"] = selv
        in_maps.append(m)
    return in_maps


def kernel(**inputs):
    from concourse.bass_utils import run_bass_kernel_spmd

    if "nc" not in _cache:
        _cache["nc"] = _build_program()
    nc = _cache["nc"]

    in_maps = _host_prep(inputs)
    res = run_bass_kernel_spmd(nc, in_maps, core_ids=list(range(N_CORES)))

    y = np.empty((B, T, C), np.float32)
    for core in range(N_CORES):
        b, j = divmod(core, N_CORES // B)
        y[b, j * TC:(j + 1) * TC] = res.results[core]["yT"].T
    return y

